# revision 32
# baseline (speedup 1.0000x reference)
"""Multi-head causal attention (B=4, S=2048, D=1024, H=16) on 8 TRN2 cores.

Sharding: core c handles batch c//2 and head-group c%2 (8 heads = 512 dims).
Each core computes its group's QKV projections, causal attention, and the
partial O-projection; the host sums the two partial outputs per batch.

v2 layout: bf16 data path (fp32 PSUM accumulation), host-prepacked inputs so
every DMA is contiguous per partition, and chunk-interleaved phases
    proj(tci) -> attn(j=tci) -> oproj(j-1)
so the PE stays dense (HAM clock at 8/8) and output DMAs overlap attention.
The ones-column of vA (softmax denominators via the AV matmul) is memset on
the vector engine, not DMA-scattered.

Attention per (j, h): scores S_T[kv, q] for kv-tile pairs into a [128,1024]
PSUM tile, diagonal mask add (DVE), one wide exp (ACT -> bf16 P), then AV
accumulates oT[65, q] with the ones column providing denominators.  AV of
pair p is issued after scores of pair p+1 (software pipeline across h and j).
Epilogue: DVE reciprocal of the denominator row, gpsimd partition-broadcast,
DVE multiply -> xT (odd heads via staging tile + DMA partition shift).
"""

import numpy as np
import ml_dtypes

import concourse.bass as bass
import concourse.mybir as mybir
import concourse.tile as tile
from concourse import bacc
from concourse.bass_utils import run_bass_kernel_spmd

F32 = mybir.dt.float32
BF16 = mybir.dt.bfloat16
EXP = mybir.ActivationFunctionType.Exp

B, S, D = 4, 2048, 1024
G = 512          # dims per head group
NT = S // 128    # 16 token tiles
NC = S // 512    # 4 token chunks
NEG = -1.0e30


def build():
    nc = bacc.Bacc("TRN2", num_devices=8)

    xq = nc.dram_tensor("xq", [128, NC * 4096], BF16, kind="ExternalInput")
    xk = nc.dram_tensor("xk", [128, NC * 4096], BF16, kind="ExternalInput")
    xv = nc.dram_tensor("xv", [128, NC * 4096], BF16, kind="ExternalInput")
    wq = nc.dram_tensor("wq", [128, 4096], BF16, kind="ExternalInput")
    wk = nc.dram_tensor("wk", [128, 4096], BF16, kind="ExternalInput")
    wv = nc.dram_tensor("wv", [128, 4096], BF16, kind="ExternalInput")
    wo = nc.dram_tensor("wo", [128, 4096], BF16, kind="ExternalInput")
    mbT_d = nc.dram_tensor("mbT", [128, 128], BF16, kind="ExternalInput")
    i128_d = nc.dram_tensor("i128", [128, 128], BF16, kind="ExternalInput")
    out_d = nc.dram_tensor("out", [S, D], F32, kind="ExternalOutput")

    with tile.TileContext(nc) as tc:
        with tc.tile_pool(name="persist", bufs=1) as persist:
            qT = persist.tile([128, 4, S], BF16, tag="qT", name="qT")
            kT = persist.tile([128, 4, S], BF16, tag="kT", name="kT")
            vA = persist.tile([128, NT, 8 * 65], BF16, tag="vA", name="vA")
            xT = persist.tile([128, 4, S], BF16, tag="xT", name="xT")
            wo_sb = persist.tile([128, 4, D], BF16, tag="wo", name="wo_sb")
            wqs = persist.tile([128, 8, G], BF16, tag="wqs", name="wqs")
            wks = persist.tile([128, 8, G], BF16, tag="wks", name="wks")
            wvs = persist.tile([128, 8, G], BF16, tag="wvs", name="wvs")
            mbT = persist.tile([128, 128], BF16, tag="mbT", name="mbT")
            i128 = persist.tile([128, 128], BF16, tag="i128", name="i128")

            # softmax-denominator ones column of vA (engine write, not a
            # 16K-element scatter DMA)
            nc.vector.memset(
                vA.rearrange("p t (h c) -> p (t h) c", c=65)[:, :, 64], 1.0
            )
            # first-matmul critical path: xq0 (issued in proj(0)) gets the
            # sync queue to itself; weights go on scalar/gpsimd
            nc.scalar.dma_start(
                out=wqs, in_=wq.ap().rearrange("p (a n) -> p a n", n=G))
            nc.scalar.dma_start(
                out=wks, in_=wk.ap().rearrange("p (a n) -> p a n", n=G))
            nc.gpsimd.dma_start(
                out=wvs, in_=wv.ap().rearrange("p (a n) -> p a n", n=G))

            with (
                tc.tile_pool(name="pxt", bufs=2) as pxt,
                tc.tile_pool(name="psb", bufs=2) as psb,
                tc.tile_pool(name="ps_acc", bufs=2, space="PSUM") as ps_acc,
                tc.tile_pool(name="ps_s", bufs=2, space="PSUM") as ps_s,
                tc.tile_pool(name="ps_o", bufs=2, space="PSUM") as ps_o,
            ):
                prev_mm = [None]

                def chain(bi):
                    # order-link attention/oproj matmuls; proj matmuls stay
                    # unchained so the scheduler can slot them into attention
                    # pipeline bubbles
                    if prev_mm[0] is not None:
                        tile.add_dep_helper(
                            bi.ins, prev_mm[0].ins, sync=False,
                            reason="PE order",
                        )
                    prev_mm[0] = bi

                def proj(tci):
                    xtq = pxt.tile([128, 8, 512], BF16, tag="xtq",
                                   name=f"xtq{tci}")
                    nc.sync.dma_start(
                        out=xtq, in_=xq.ap()[:, 4096 * tci:4096 * tci + 4096]
                        .rearrange("p (a t) -> p a t", t=512))
                    xtk = pxt.tile([128, 8, 512], BF16, tag="xtk",
                                   name=f"xtk{tci}")
                    nc.gpsimd.dma_start(
                        out=xtk, in_=xk.ap()[:, 4096 * tci:4096 * tci + 4096]
                        .rearrange("p (a t) -> p a t", t=512))
                    xtv = pxt.tile([128, 8, 512], BF16, tag="xtv",
                                   name=f"xtv{tci}")
                    nc.scalar.dma_start(
                        out=xtv, in_=xv.ap()[:, 4096 * tci:4096 * tci + 4096]
                        .rearrange("p (a t) -> p a t", t=512))
                    with nc.named_scope("proj"):
                        for kind, xt, wsb, dest in (
                            ("q", xtq, wqs, qT), ("k", xtk, wks, kT),
                        ):
                            for dq in range(4):
                                acc = ps_acc.tile([128, 512], F32, tag="acc",
                                                  name=f"pj{kind}{tci}{dq}")
                                for dm in range(8):
                                    nc.tensor.matmul(
                                        acc,
                                        wsb[:, dm, 128 * dq:128 * dq + 128],
                                        xt[:, dm, :],
                                        start=(dm == 0), stop=(dm == 7),
                                    )
                                nc.vector.tensor_copy(
                                    out=dest[:, dq, 512 * tci:512 * tci + 512],
                                    in_=acc,
                                )
                        for tt in range(4):
                            gtt = 4 * tci + tt
                            acc = ps_acc.tile([128, 512], F32, tag="acc",
                                              name=f"pjv{gtt}")
                            for dm in range(8):
                                nc.tensor.matmul(
                                    acc,
                                    xtv[:, dm, 128 * tt:128 * tt + 128],
                                    wvs[:, dm, :],
                                    start=(dm == 0), stop=(dm == 7),
                                )
                            nc.vector.tensor_copy(
                                out=vA[:, gtt, :]
                                .rearrange("p (h c) -> p h c", c=65)[:, :, 0:64],
                                in_=acc.rearrange("p (h c) -> p h c", c=64),
                            )

                pending = [None]  # (j, h, oT, p, pt, q00, q01)

                def av_pending():
                    j, h, oT, p, pt, q00, q01 = pending[0]
                    pending[0] = None
                    kv0, kv1 = 2 * p, 2 * p + 1
                    last = 4 * j + 3
                    chain(nc.tensor.matmul(
                        oT[:, q00:512],
                        vA[:, kv0, 65 * h:65 * h + 65],
                        pt[:, q00:512],
                        start=(kv0 == 0), stop=(kv0 == last),
                    ))
                    chain(nc.tensor.matmul(
                        oT[:, q01:512],
                        vA[:, kv1, 65 * h:65 * h + 65],
                        pt[:, 512 + q01:1024],
                        start=False, stop=(kv1 == last),
                    ))
                    if p == 2 * j + 1:
                        epilogue(j, h, oT)

                def epilogue(j, h, oT):
                    # copy O and DMA the denominator row out of PSUM first
                    # (frees the oT bank in ~1us); reciprocal runs
                    # partition-aligned at p0 (custom DVE ops cannot cross
                    # partitions, unlike InstReciprocal)
                    d, off = h // 2, 64 * (h % 2)
                    xU = psb.tile([65, 512], F32, tag="xU", bufs=3,
                                  name=f"xU_{j}_{h}")
                    nc.vector.tensor_copy(out=xU, in_=oT[0:65, :])
                    den0 = psb.tile([1, 512], F32, tag="den", bufs=2,
                                    name=f"den_{j}_{h}")
                    nc.gpsimd.dma_start(out=den0, in_=xU[64:65, :])
                    rrow = psb.tile([1, 512], F32, tag="rrow", bufs=2,
                                    name=f"rrow_{j}_{h}")
                    nc.vector.reciprocal_approx_fast(out=rrow, in_=den0)
                    rbc = psb.tile([64, 512], F32, tag="rbc", bufs=2,
                                   name=f"rbc_{j}_{h}")
                    nc.gpsimd.partition_broadcast(rbc, rrow, channels=64)
                    if off == 0:
                        nc.vector.tensor_mul(
                            xT[0:64, d, 512 * j:512 * j + 512],
                            xU[0:64, :], rbc,
                        )
                    else:
                        xtmp = psb.tile([64, 512], BF16, tag="xtmp", bufs=2,
                                        name=f"xtmp_{j}_{h}")
                        nc.vector.tensor_mul(xtmp, xU[0:64, :], rbc)
                        nc.gpsimd.dma_start(
                            out=xT[64:128, d, 512 * j:512 * j + 512],
                            in_=xtmp,
                        )

                def attn_chunk(j, po_work=None):
                    with nc.named_scope("attn"):
                        for h in range(8):
                            d, off = h // 2, 64 * (h % 2)
                            kTh = kT[off:off + 64, d, :]
                            qTh = qT[off:off + 64, d, :]
                            oT = ps_o.tile([65, 512], F32, tag="O",
                                           name=f"oT_{j}_{h}")
                            for p in range(2 * j + 2):
                                kv0, kv1 = 2 * p, 2 * p + 1
                                q00 = max(0, 128 * kv0 - 512 * j)
                                q01 = max(0, 128 * kv1 - 512 * j)
                                sbig = ps_s.tile([128, 1024], F32, tag="S",
                                                 name=f"s_{j}_{h}_{p}")
                                # diagonal tiles get the causal mask added by
                                # the PE itself (mbT.T @ I accumulated into
                                # the 128-wide diag block) — keeps the DVE
                                # out of the scores->exp critical path
                                for half, kv, qq in ((0, kv0, q00),
                                                     (1, kv1, q01)):
                                    diag = kv >= 4 * j
                                    chain(nc.tensor.matmul(
                                        sbig[:, 512 * half + qq:
                                             512 * half + 512],
                                        kTh[:, 128 * kv:128 * kv + 128],
                                        qTh[:, 512 * j + qq:512 * j + 512],
                                        start=True, stop=not diag,
                                    ))
                                    if diag:
                                        c0 = 512 * half + qq
                                        chain(nc.tensor.matmul(
                                            sbig[:, c0:c0 + 128],
                                            mbT, i128,
                                            start=False, stop=True,
                                        ))
                                # one exp per pair; cols [512, 512+q01) are
                                # never-written PSUM whose exp lands in an
                                # unused pt range (AV reads [512+q01:) only)
                                pt = psb.tile([128, 1024], BF16, tag="pt",
                                              bufs=6, name=f"pt_{j}_{h}_{p}")
                                nc.scalar.activation(
                                    pt[:, q00:1024], sbig[:, q00:1024],
                                    EXP, scale=0.125)
                                if pending[0] is not None:
                                    av_pending()
                                pending[0] = (j, h, oT, p, pt, q00, q01)
                            if po_work is not None and h % 2 == 1:
                                oproj_tile(po_work[(h - 1) // 2])
                        # flush so xT chunk j completes before oproj(j)
                        av_pending()

                def oproj_tile(i, tail=False):
                    # one 128-token output tile; po reuses the proj acc PSUM
                    # slots (idle during attention).  Tail tiles copy on the
                    # by-then-idle ScalarE (DVE is draining the last
                    # epilogues)
                    with nc.named_scope("oproj"):
                        po = [
                            ps_acc.tile([128, 512], F32, tag="acc",
                                        name=f"po_{i}_{n}")
                            for n in range(2)
                        ]
                        for dd in range(4):
                            for n in range(2):
                                chain(nc.tensor.matmul(
                                    po[n],
                                    xT[:, dd, 128 * i:128 * i + 128],
                                    wo_sb[:, dd, 512 * n:512 * n + 512],
                                    start=(dd == 0), stop=(dd == 3),
                                ))
                        for n in range(2):
                            ob = psb.tile([128, 512], F32, tag="ob",
                                          bufs=3, name=f"ob_{i}_{n}")
                            if tail:
                                nc.scalar.copy(ob, po[n])
                            else:
                                nc.vector.tensor_copy(out=ob, in_=po[n])
                            nc.sync.dma_start(
                                out=out_d.ap()[128 * i:128 * i + 128,
                                               512 * n:512 * n + 512],
                                in_=ob,
                            )

                # chunk-interleaved schedule: oproj(j-1) po-groups are spread
                # between attn(j) heads as PE filler during the ACT-bound
                # attention stretches; epilogues drain a full chunk before
                # their xT is consumed
                proj(0)
                nc.scalar.dma_start(out=mbT, in_=mbT_d.ap())
                nc.scalar.dma_start(out=i128, in_=i128_d.ap())
                nc.scalar.dma_start(
                    out=wo_sb, in_=wo.ap().rearrange("p (a n) -> p a n", n=D))
                attn_chunk(0)
                proj(1)
                attn_chunk(1, po_work=[0, 1, 2, 3])
                proj(2)
                attn_chunk(2, po_work=[4, 5, 6, 7])
                proj(3)
                attn_chunk(3, po_work=[8, 9, 10, 11])
                for i in (12, 13, 14, 15):
                    oproj_tile(i, tail=True)

    nc.compile()
    return nc


_NC = None


def _get_nc():
    global _NC
    if _NC is None:
        _NC = build()
    return _NC


def _pack_x(xb):
    # [S, D] f32 -> [128, NC*4096] bf16; [p, (tci a s)] = xb[512 tci + s, 128 a + p]
    xt = np.ascontiguousarray(np.asarray(xb, np.float32).T)  # [D, S]
    v = xt.reshape(8, 128, NC, 512).transpose(1, 2, 0, 3)    # [p, tci, a, s]
    return np.ascontiguousarray(v.reshape(128, NC * 4096)).astype(
        ml_dtypes.bfloat16)


def _pack_w(w, g):
    # rows [G g, G g + G) of w, transposed: [p, (a n)] = w[G g + n, 128 a + p]
    wgT = np.ascontiguousarray(np.asarray(w, np.float32)[G * g:G * g + G, :].T)
    v = wgT.reshape(8, 128, G).transpose(1, 0, 2)
    return np.ascontiguousarray(v.reshape(128, 8 * G)).astype(
        ml_dtypes.bfloat16)


def _pack_wo(w_o, g):
    # cols [G g, G g + G) of w_o, transposed: [p, (a n)] = w_o[n, G g + 128 a + p]
    woT = np.ascontiguousarray(
        np.asarray(w_o, np.float32)[:, G * g:G * g + G].T)  # [G, D]
    v = woT.reshape(4, 128, D).transpose(1, 0, 2)
    return np.ascontiguousarray(v.reshape(128, 4 * D)).astype(
        ml_dtypes.bfloat16)


def _make_in_maps(q, k, v, w_q, w_k, w_v, w_o):
    # additive causal mask for a delta=0 diagonal 128x128 tile, applied by
    # the PE as mbT.T @ I: S_T[kl, ql] masked (-> -1e30) iff ql < kl, so
    # mbT[ql, kl] = 0 if ql >= kl else -1e30
    col = np.arange(128)[None, :]
    row = np.arange(128)[:, None]
    mbT = np.where(row >= col, 0.0, NEG).astype(ml_dtypes.bfloat16)
    i128 = np.eye(128).astype(ml_dtypes.bfloat16)

    xqp = [_pack_x(q[b]) for b in range(B)]
    xkp = [_pack_x(k[b]) for b in range(B)]
    xvp = [_pack_x(v[b]) for b in range(B)]
    wqp = [_pack_w(w_q, g) for g in range(2)]
    wkp = [_pack_w(w_k, g) for g in range(2)]
    wvp = [_pack_w(w_v, g) for g in range(2)]
    wop = [_pack_wo(w_o, g) for g in range(2)]

    in_maps = []
    for c in range(8):
        b, g = c // 2, c % 2
        in_maps.append({
            "xq": xqp[b], "xk": xkp[b], "xv": xvp[b],
            "wq": wqp[g], "wk": wkp[g], "wv": wvp[g], "wo": wop[g],
            "mbT": mbT, "i128": i128,
        })
    return in_maps


def _gather(results):
    out = np.empty((B, S, D), np.float32)
    for b in range(B):
        out[b] = results[2 * b]["out"] + results[2 * b + 1]["out"]
    return out


def run_kernel(inputs, trace=False, tmpdir=None):
    """Run on 8 cores; returns (out, BassKernelResults)."""
    in_maps = _make_in_maps(
        inputs["q"], inputs["k"], inputs["v"],
        inputs["w_q"], inputs["w_k"], inputs["w_v"], inputs["w_o"],
    )
    res = run_bass_kernel_spmd(
        _get_nc(), in_maps, core_ids=list(range(8)), trace=trace, tmpdir=tmpdir
    )
    return _gather(res.results), res


def kernel(**inputs) -> np.ndarray:
    out, _ = run_kernel(inputs)
    return out


# revision 36
# speedup vs baseline: 1.2215x; 1.2215x over previous
"""Multi-head causal attention (B=4, S=2048, D=1024, H=16) on 8 TRN2 cores.

Sharding: core c handles batch c//2 and head-group c%2 (8 heads = 512 dims).
Each core computes its group's QKV projections, causal attention, and the
partial O-projection; the host sums the two partial outputs per batch.

v2 layout: bf16 data path (fp32 PSUM accumulation), host-prepacked inputs so
every DMA is contiguous per partition, and chunk-interleaved phases
    proj(tci) -> attn(j=tci) -> oproj(j-1)
so the PE stays dense (HAM clock at 8/8) and output DMAs overlap attention.
The ones-column of vA (softmax denominators via the AV matmul) is memset on
the vector engine, not DMA-scattered.

Attention per (j, h): scores S_T[kv, q] for kv-tile pairs into a [128,1024]
PSUM tile, diagonal mask add (DVE), one wide exp (ACT -> bf16 P), then AV
accumulates oT[65, q] with the ones column providing denominators.  AV of
pair p is issued after scores of pair p+1 (software pipeline across h and j).
Epilogue: DVE reciprocal of the denominator row, gpsimd partition-broadcast,
DVE multiply -> xT (odd heads via staging tile + DMA partition shift).
"""

import numpy as np
import ml_dtypes

import concourse.bass as bass
import concourse.mybir as mybir
import concourse.tile as tile
from concourse import bacc
from concourse.bass_utils import run_bass_kernel_spmd

F32 = mybir.dt.float32
BF16 = mybir.dt.bfloat16
EXP = mybir.ActivationFunctionType.Exp

B, S, D = 4, 2048, 1024
G = 512          # dims per head group
NT = S // 128    # 16 token tiles
NC = S // 512    # 4 token chunks
NEG = -1.0e30


def build():
    nc = bacc.Bacc("TRN2", num_devices=8)

    xq = nc.dram_tensor("xq", [128, NC * 4096], BF16, kind="ExternalInput")
    xk = nc.dram_tensor("xk", [128, NC * 4096], BF16, kind="ExternalInput")
    xv = nc.dram_tensor("xv", [128, NC * 4096], BF16, kind="ExternalInput")
    wq = nc.dram_tensor("wq", [128, 4096], BF16, kind="ExternalInput")
    wk = nc.dram_tensor("wk", [128, 4096], BF16, kind="ExternalInput")
    wv = nc.dram_tensor("wv", [128, 4096], BF16, kind="ExternalInput")
    wo = nc.dram_tensor("wo", [128, 4096], BF16, kind="ExternalInput")
    mbT_d = nc.dram_tensor("mbT", [128, 128], BF16, kind="ExternalInput")
    i128_d = nc.dram_tensor("i128", [128, 128], BF16, kind="ExternalInput")
    out_d = nc.dram_tensor("out", [S, D], F32, kind="ExternalOutput")

    with tile.TileContext(nc) as tc:
        with tc.tile_pool(name="persist", bufs=1) as persist:
            qT = persist.tile([128, 4, S], BF16, tag="qT", name="qT")
            kT = persist.tile([128, 4, S], BF16, tag="kT", name="kT")
            vA = persist.tile([128, NT, 8 * 65], BF16, tag="vA", name="vA")
            xT = persist.tile([128, 4, S], BF16, tag="xT", name="xT")
            wo_sb = persist.tile([128, 4, D], BF16, tag="wo", name="wo_sb")
            wqs = persist.tile([128, 8, G], BF16, tag="wqs", name="wqs")
            wks = persist.tile([128, 8, G], BF16, tag="wks", name="wks")
            wvs = persist.tile([128, 8, G], BF16, tag="wvs", name="wvs")
            mbT = persist.tile([128, 128], BF16, tag="mbT", name="mbT")
            i128 = persist.tile([128, 128], BF16, tag="i128", name="i128")

            # softmax-denominator ones column of vA (engine write, not a
            # 16K-element scatter DMA)
            nc.vector.memset(
                vA.rearrange("p t (h c) -> p (t h) c", c=65)[:, :, 64], 1.0
            )
            # first-matmul critical path: wq on sync, xq0 (issued in proj(0))
            # leads the scalar queue; wo is not needed until oproj(0)
            nc.sync.dma_start(
                out=wqs, in_=wq.ap().rearrange("p (a n) -> p a n", n=G))
            nc.sync.dma_start(
                out=wks, in_=wk.ap().rearrange("p (a n) -> p a n", n=G))
            nc.gpsimd.dma_start(
                out=wvs, in_=wv.ap().rearrange("p (a n) -> p a n", n=G))

            with (
                tc.tile_pool(name="pxt", bufs=2) as pxt,
                tc.tile_pool(name="psb", bufs=2) as psb,
                tc.tile_pool(name="ps_acc", bufs=2, space="PSUM") as ps_acc,
                tc.tile_pool(name="ps_s", bufs=2, space="PSUM") as ps_s,
                tc.tile_pool(name="ps_o", bufs=2, space="PSUM") as ps_o,
            ):
                prev_mm = [None]

                def chain(bi):
                    # order-link attention/oproj matmuls; proj matmuls stay
                    # unchained so the scheduler can slot them into attention
                    # pipeline bubbles
                    if prev_mm[0] is not None:
                        tile.add_dep_helper(
                            bi.ins, prev_mm[0].ins, sync=False,
                            reason="PE order",
                        )
                    prev_mm[0] = bi

                def proj(tci):
                    xtq = pxt.tile([128, 8, 512], BF16, tag="xtq",
                                   name=f"xtq{tci}")
                    nc.scalar.dma_start(
                        out=xtq, in_=xq.ap()[:, 4096 * tci:4096 * tci + 4096]
                        .rearrange("p (a t) -> p a t", t=512))
                    xtk = pxt.tile([128, 8, 512], BF16, tag="xtk",
                                   name=f"xtk{tci}")
                    nc.gpsimd.dma_start(
                        out=xtk, in_=xk.ap()[:, 4096 * tci:4096 * tci + 4096]
                        .rearrange("p (a t) -> p a t", t=512))
                    xtv = pxt.tile([128, 8, 512], BF16, tag="xtv",
                                   name=f"xtv{tci}")
                    nc.sync.dma_start(
                        out=xtv, in_=xv.ap()[:, 4096 * tci:4096 * tci + 4096]
                        .rearrange("p (a t) -> p a t", t=512))
                    with nc.named_scope("proj"):
                        for kind, xt, wsb, dest in (
                            ("q", xtq, wqs, qT), ("k", xtk, wks, kT),
                        ):
                            for dq in range(4):
                                acc = ps_acc.tile([128, 512], F32, tag="acc",
                                                  name=f"pj{kind}{tci}{dq}")
                                for dm in range(8):
                                    nc.tensor.matmul(
                                        acc,
                                        wsb[:, dm, 128 * dq:128 * dq + 128],
                                        xt[:, dm, :],
                                        start=(dm == 0), stop=(dm == 7),
                                    )
                                nc.vector.tensor_copy(
                                    out=dest[:, dq, 512 * tci:512 * tci + 512],
                                    in_=acc,
                                )
                        for tt in range(4):
                            gtt = 4 * tci + tt
                            acc = ps_acc.tile([128, 512], F32, tag="acc",
                                              name=f"pjv{gtt}")
                            for dm in range(8):
                                nc.tensor.matmul(
                                    acc,
                                    xtv[:, dm, 128 * tt:128 * tt + 128],
                                    wvs[:, dm, :],
                                    start=(dm == 0), stop=(dm == 7),
                                )
                            nc.vector.tensor_copy(
                                out=vA[:, gtt, :]
                                .rearrange("p (h c) -> p h c", c=65)[:, :, 0:64],
                                in_=acc.rearrange("p (h c) -> p h c", c=64),
                            )

                pending = [None]  # (j, h, oT, p, pt, q00, q01)

                def av_pending():
                    j, h, oT, p, pt, q00, q01 = pending[0]
                    pending[0] = None
                    kv0, kv1 = 2 * p, 2 * p + 1
                    last = 4 * j + 3
                    chain(nc.tensor.matmul(
                        oT[:, q00:512],
                        vA[:, kv0, 65 * h:65 * h + 65],
                        pt[:, q00:512],
                        start=(kv0 == 0), stop=(kv0 == last),
                    ))
                    chain(nc.tensor.matmul(
                        oT[:, q01:512],
                        vA[:, kv1, 65 * h:65 * h + 65],
                        pt[:, 512 + q01:1024],
                        start=False, stop=(kv1 == last),
                    ))
                    if p == 2 * j + 1:
                        epilogue(j, h, oT)

                def epilogue(j, h, oT):
                    # copy O and DMA the denominator row out of PSUM first
                    # (frees the oT bank in ~1us); reciprocal runs
                    # partition-aligned at p0 (custom DVE ops cannot cross
                    # partitions, unlike InstReciprocal)
                    d, off = h // 2, 64 * (h % 2)
                    xU = psb.tile([65, 512], F32, tag="xU", bufs=3,
                                  name=f"xU_{j}_{h}")
                    nc.vector.tensor_copy(out=xU, in_=oT[0:65, :])
                    den0 = psb.tile([1, 512], F32, tag="den", bufs=2,
                                    name=f"den_{j}_{h}")
                    nc.gpsimd.dma_start(out=den0, in_=xU[64:65, :])
                    rrow = psb.tile([1, 512], F32, tag="rrow", bufs=2,
                                    name=f"rrow_{j}_{h}")
                    nc.vector.reciprocal_approx_fast(out=rrow, in_=den0)
                    rbc = psb.tile([64, 512], F32, tag="rbc", bufs=2,
                                   name=f"rbc_{j}_{h}")
                    nc.gpsimd.partition_broadcast(rbc, rrow, channels=64)
                    if off == 0:
                        nc.vector.tensor_mul(
                            xT[0:64, d, 512 * j:512 * j + 512],
                            xU[0:64, :], rbc,
                        )
                    else:
                        xtmp = psb.tile([64, 512], BF16, tag="xtmp", bufs=2,
                                        name=f"xtmp_{j}_{h}")
                        nc.vector.tensor_mul(xtmp, xU[0:64, :], rbc)
                        nc.gpsimd.dma_start(
                            out=xT[64:128, d, 512 * j:512 * j + 512],
                            in_=xtmp,
                        )

                def attn_chunk(j, po_work=None):
                    with nc.named_scope("attn"):
                        for h in range(8):
                            d, off = h // 2, 64 * (h % 2)
                            kTh = kT[off:off + 64, d, :]
                            qTh = qT[off:off + 64, d, :]
                            oT = ps_o.tile([65, 512], F32, tag="O",
                                           name=f"oT_{j}_{h}")
                            for p in range(2 * j + 2):
                                kv0, kv1 = 2 * p, 2 * p + 1
                                q00 = max(0, 128 * kv0 - 512 * j)
                                q01 = max(0, 128 * kv1 - 512 * j)
                                sbig = ps_s.tile([128, 1024], F32, tag="S",
                                                 name=f"s_{j}_{h}_{p}")
                                # diagonal tiles get the causal mask added by
                                # the PE itself (mbT.T @ I accumulated into
                                # the 128-wide diag block) — keeps the DVE
                                # out of the scores->exp critical path
                                for half, kv, qq in ((0, kv0, q00),
                                                     (1, kv1, q01)):
                                    diag = kv >= 4 * j
                                    chain(nc.tensor.matmul(
                                        sbig[:, 512 * half + qq:
                                             512 * half + 512],
                                        kTh[:, 128 * kv:128 * kv + 128],
                                        qTh[:, 512 * j + qq:512 * j + 512],
                                        start=True, stop=not diag,
                                    ))
                                    if diag:
                                        c0 = 512 * half + qq
                                        chain(nc.tensor.matmul(
                                            sbig[:, c0:c0 + 128],
                                            mbT, i128,
                                            start=False, stop=True,
                                        ))
                                pt = psb.tile([128, 1024], BF16, tag="pt",
                                              bufs=6, name=f"pt_{j}_{h}_{p}")
                                if q00 > 0 or q01 > 0:
                                    nc.scalar.activation(
                                        pt[:, q00:512], sbig[:, q00:512],
                                        EXP, scale=0.125)
                                    nc.scalar.activation(
                                        pt[:, 512 + q01:1024],
                                        sbig[:, 512 + q01:1024],
                                        EXP, scale=0.125)
                                else:
                                    nc.scalar.activation(
                                        pt, sbig, EXP, scale=0.125)
                                if pending[0] is not None:
                                    av_pending()
                                pending[0] = (j, h, oT, p, pt, q00, q01)
                            if po_work is not None and h % 2 == 1:
                                oproj_tile(po_work[(h - 1) // 2])
                        # flush so xT chunk j completes before oproj(j)
                        av_pending()

                def oproj_tile(i, tail=False):
                    # one 128-token output tile; po reuses the proj acc PSUM
                    # slots (idle during attention).  Tail tiles copy on the
                    # by-then-idle ScalarE (DVE is draining the last
                    # epilogues)
                    with nc.named_scope("oproj"):
                        po = [
                            ps_acc.tile([128, 512], F32, tag="acc",
                                        name=f"po_{i}_{n}")
                            for n in range(2)
                        ]
                        for dd in range(4):
                            for n in range(2):
                                chain(nc.tensor.matmul(
                                    po[n],
                                    xT[:, dd, 128 * i:128 * i + 128],
                                    wo_sb[:, dd, 512 * n:512 * n + 512],
                                    start=(dd == 0), stop=(dd == 3),
                                ))
                        for n in range(2):
                            ob = psb.tile([128, 512], F32, tag="ob",
                                          bufs=3, name=f"ob_{i}_{n}")
                            if tail:
                                nc.scalar.copy(ob, po[n])
                            else:
                                nc.vector.tensor_copy(out=ob, in_=po[n])
                            nc.sync.dma_start(
                                out=out_d.ap()[128 * i:128 * i + 128,
                                               512 * n:512 * n + 512],
                                in_=ob,
                            )

                # chunk-interleaved schedule: oproj(j-1) po-groups are spread
                # between attn(j) heads as PE filler during the ACT-bound
                # attention stretches; epilogues drain a full chunk before
                # their xT is consumed
                proj(0)
                nc.scalar.dma_start(out=mbT, in_=mbT_d.ap())
                nc.scalar.dma_start(out=i128, in_=i128_d.ap())
                nc.scalar.dma_start(
                    out=wo_sb, in_=wo.ap().rearrange("p (a n) -> p a n", n=D))
                attn_chunk(0)
                proj(1)
                attn_chunk(1, po_work=[0, 1, 2, 3])
                proj(2)
                attn_chunk(2, po_work=[4, 5, 6, 7])
                proj(3)
                attn_chunk(3, po_work=[8, 9, 10, 11])
                for i in (12, 13, 14, 15):
                    oproj_tile(i, tail=True)

    nc.compile()
    return nc


_NC = None


def _get_nc():
    global _NC
    if _NC is None:
        _NC = build()
    return _NC


def _pack_x(xb):
    # [S, D] f32 -> [128, NC*4096] bf16; [p, (tci a s)] = xb[512 tci + s, 128 a + p]
    xt = np.ascontiguousarray(np.asarray(xb, np.float32).T)  # [D, S]
    v = xt.reshape(8, 128, NC, 512).transpose(1, 2, 0, 3)    # [p, tci, a, s]
    return np.ascontiguousarray(v.reshape(128, NC * 4096)).astype(
        ml_dtypes.bfloat16)


def _pack_w(w, g):
    # rows [G g, G g + G) of w, transposed: [p, (a n)] = w[G g + n, 128 a + p]
    wgT = np.ascontiguousarray(np.asarray(w, np.float32)[G * g:G * g + G, :].T)
    v = wgT.reshape(8, 128, G).transpose(1, 0, 2)
    return np.ascontiguousarray(v.reshape(128, 8 * G)).astype(
        ml_dtypes.bfloat16)


def _pack_wo(w_o, g):
    # cols [G g, G g + G) of w_o, transposed: [p, (a n)] = w_o[n, G g + 128 a + p]
    woT = np.ascontiguousarray(
        np.asarray(w_o, np.float32)[:, G * g:G * g + G].T)  # [G, D]
    v = woT.reshape(4, 128, D).transpose(1, 0, 2)
    return np.ascontiguousarray(v.reshape(128, 4 * D)).astype(
        ml_dtypes.bfloat16)


def _make_in_maps(q, k, v, w_q, w_k, w_v, w_o):
    # additive causal mask for a delta=0 diagonal 128x128 tile, applied by
    # the PE as mbT.T @ I: S_T[kl, ql] masked (-> -1e30) iff ql < kl, so
    # mbT[ql, kl] = 0 if ql >= kl else -1e30
    col = np.arange(128)[None, :]
    row = np.arange(128)[:, None]
    mbT = np.where(row >= col, 0.0, NEG).astype(ml_dtypes.bfloat16)
    i128 = np.eye(128).astype(ml_dtypes.bfloat16)

    xqp = [_pack_x(q[b]) for b in range(B)]
    xkp = [_pack_x(k[b]) for b in range(B)]
    xvp = [_pack_x(v[b]) for b in range(B)]
    wqp = [_pack_w(w_q, g) for g in range(2)]
    wkp = [_pack_w(w_k, g) for g in range(2)]
    wvp = [_pack_w(w_v, g) for g in range(2)]
    wop = [_pack_wo(w_o, g) for g in range(2)]

    in_maps = []
    for c in range(8):
        b, g = c // 2, c % 2
        in_maps.append({
            "xq": xqp[b], "xk": xkp[b], "xv": xvp[b],
            "wq": wqp[g], "wk": wkp[g], "wv": wvp[g], "wo": wop[g],
            "mbT": mbT, "i128": i128,
        })
    return in_maps


def _gather(results):
    out = np.empty((B, S, D), np.float32)
    for b in range(B):
        out[b] = results[2 * b]["out"] + results[2 * b + 1]["out"]
    return out


def run_kernel(inputs, trace=False, tmpdir=None):
    """Run on 8 cores; returns (out, BassKernelResults)."""
    in_maps = _make_in_maps(
        inputs["q"], inputs["k"], inputs["v"],
        inputs["w_q"], inputs["w_k"], inputs["w_v"], inputs["w_o"],
    )
    res = run_bass_kernel_spmd(
        _get_nc(), in_maps, core_ids=list(range(8)), trace=trace, tmpdir=tmpdir
    )
    return _gather(res.results), res


def kernel(**inputs) -> np.ndarray:
    out, _ = run_kernel(inputs)
    return out
